# revision 21
# baseline (speedup 1.0000x reference)
"""AttnBlock2D Trainium2 kernel (8-core data-parallel over batch).

Per core: one batch element. x:[512, 4096] (c, h*w).
  h = GroupNorm32(x) * scale + bias
  q = wq@h, k = wk@h, v = wv@h          (1x1 conv == matmul over channels)
  attn = softmax(q^T k / sqrt(512));  out = v @ attn^T
  y = x + wp@out + bp'                  (bp' = bp + wp@bv, folded on host)

On-chip layout: S^T orientation ([keys on partitions, queries on free]) so the
softmax denominator is a partition-dim reduction done with a ones-matmul; the
PV contraction then needs no transposes at all (vT is produced directly by
swapping matmul operands). Softmax skips max-subtraction: scores are ~N(0,1)
by construction; exp() gets a -2 bias so fp8e4m3 outputs can't overflow (the
factor cancels between numerator and denominator).

All big matmuls run fp8e4m3 with perf_mode=DoubleRow: operands are stored as
[128, 2, n] (contraction = 128 partitions x 2 interleaved k-tiles), which
halves instruction count and PE streaming time vs bf16. PSUM stays fp32.
Attention-path quantization error is diluted ~40x in the output norm by the
residual, so rel-err stays ~5e-3, under the 2e-2 gate.

Softmax normalization is applied LATE (after the output projection):
y = x + (wp@U)*r + bp' with U the unnormalized PV output and r = 1/denominator
per query. This lets U leave PSUM as plain copies right after the last PV
(freeing the PV accumulator banks for the next query block immediately), and
the projection matmuls + normalize are emitted as pending stages interleaved
into the next query block's S-loop so the PE never waits on the recip chain.
"""
import os
import numpy as np

P = 128
C = 512
NCH = C // P              # 4 channel chunks
NC2 = NCH // 2            # 2 channel pair-chunks (DoubleRow)
HW = 4096                 # 64*64 pixels
QB = 512                  # query block (PSUM-bank limited)
NQB = HW // QB            # 8
NE = HW // P              # 32 key chunks
NE2 = NE // 2             # 16 key pair-chunks
NB = HW // 512            # 8 pixel blocks for projections
EPS = 1e-5
SCALE = 1.0 / np.sqrt(C)
# exp bias: scores are ~N(0,1), so exp(s - 3.39) keeps the fp8 probabilities
# and (more importantly) the unnormalized PV output U = sum(pt*v) safely
# below e4m3's 240 max while staying above the subnormal floor for all but
# a ~4% tail. The constant cancels between softmax numerator & denominator.
EXPB = -2.0 - float(np.log(4.0))
B = 8                     # batch / cores

_CACHE = {}


def _emit(nc, tc, ctx):
    import concourse.bass as bass
    from concourse import mybir
    from contextlib import ExitStack

    f32 = mybir.dt.float32
    bf16 = mybir.dt.bfloat16
    f8 = mybir.dt.float8e4
    AF = mybir.ActivationFunctionType
    OP = mybir.AluOpType
    DR = mybir.MatmulPerfMode.DoubleRow

    # ---------------- I/O ----------------
    x_d = nc.declare_dram_parameter("x", [C, HW], f32, isOutput=False).ap()
    wqT_d = nc.declare_dram_parameter("wqT", [C, C], f32, isOutput=False).ap()
    wkT_d = nc.declare_dram_parameter("wkT", [C, C], f32, isOutput=False).ap()
    wvT_d = nc.declare_dram_parameter("wvT", [C, C], f32, isOutput=False).ap()
    wpT_d = nc.declare_dram_parameter("wpT", [C, C], f32, isOutput=False).ap()
    bq_d = nc.declare_dram_parameter("bq", [C], f32, isOutput=False).ap()
    bk_d = nc.declare_dram_parameter("bk", [C], f32, isOutput=False).ap()
    bp_d = nc.declare_dram_parameter("bp", [C], f32, isOutput=False).ap()
    ns_d = nc.declare_dram_parameter("nscale", [C], f32, isOutput=False).ap()
    nb_d = nc.declare_dram_parameter("nbias", [C], f32, isOutput=False).ap()
    i16_d = nc.declare_dram_parameter("ind16", [P, 8], f32, isOutput=False).ap()
    iT_d = nc.declare_dram_parameter("indT", [8, P], f32, isOutput=False).ap()
    out_d = nc.declare_dram_parameter("out", [C, HW], f32, isOutput=True).ap()

    def col_ap(src, ci):
        # [128] slice of a [512] DRAM vector viewed as [128, 1]
        return bass.AP(tensor=src.tensor, offset=ci * P, ap=[[1, P], [0, 1]])

    # ---------------- persistent pools (allocated before phase-1a scope) ----
    cst = ctx.enter_context(tc.tile_pool(name="cst", bufs=1))
    wpT_sb = [cst.tile([P, 2, C], f8, name=f"wpT{i}", tag=f"wpT{i}")
              for i in range(NC2)]
    ind16_sb = cst.tile([P, 8], f32, name="ind16", tag="ind16")
    indT_sb = cst.tile([8, P], f32, name="indT", tag="indT")
    onec_sb = cst.tile([P, 1], bf16, name="onec", tag="onec")
    oner_sb = cst.tile([1, P], bf16, name="oner", tag="oner")
    bp_sb = [cst.tile([P, 1], f32, name=f"bp{i}", tag=f"bp{i}") for i in range(NCH)]
    expb_sb = cst.tile([P, 1], f32, name="expb", tag="expb")

    w_pool = tc.alloc_tile_pool(name="wqkv", bufs=1, side="right")
    wq2 = [w_pool.tile([P, 2, C], f8, name=f"wq2{i}", tag=f"wq2{i}") for i in range(NC2)]
    wk2 = [w_pool.tile([P, 2, C], f8, name=f"wk2{i}", tag=f"wk2{i}") for i in range(NC2)]
    wv2 = [w_pool.tile([P, 2, C], f8, name=f"wv2{i}", tag=f"wv2{i}") for i in range(NC2)]
    bq_sb = [w_pool.tile([P, 1], f32, name=f"bq{i}", tag=f"bq{i}") for i in range(NCH)]
    bk_sb = [w_pool.tile([P, 1], f32, name=f"bk{i}", tag=f"bk{i}") for i in range(NCH)]

    h_pool = tc.alloc_tile_pool(name="hres", bufs=1, side="right")
    h2 = [h_pool.tile([P, 2, HW], f8, name=f"h{i}", tag=f"h{i}") for i in range(NC2)]

    # ================ phase 1a: groupnorm (x DMA emitted first) ============
    with ExitStack() as s1:
        xp = s1.enter_context(tc.tile_pool(name="xp", bufs=1))
        wraw = s1.enter_context(tc.tile_pool(name="wraw", bufs=4))
        gn = s1.enter_context(tc.tile_pool(name="gn", bufs=2))
        gnp = s1.enter_context(tc.tile_pool(name="gnp", bufs=2, space="PSUM"))

        # x first: it gates the whole head (stats -> h -> qkv). Two DMAs per
        # chunk so per-block bn_stats can start on the first half early.
        x_sb = []
        for ci in range(NCH):
            t = xp.tile([P, HW], f32, name=f"x{ci}", tag=f"x{ci}")
            nc.sync.dma_start(out=t[:, 0:HW // 2],
                              in_=x_d[ci * P:(ci + 1) * P, 0:HW // 2])
            nc.sync.dma_start(out=t[:, HW // 2:HW],
                              in_=x_d[ci * P:(ci + 1) * P, HW // 2:HW])
            x_sb.append(t)
        # PE warmup: the HAM clock gate keeps the PE at 1.2 GHz until it has
        # been busy ~3.4us; burn dummy matmuls during the DMA/stats head so
        # phase 1b starts at 2.4 GHz.
        wrm = gn.tile([P, 512], f32, name="wrm", tag="wrm", bufs=1)
        nc.vector.memset(wrm, 1.0)
        wrm_ps = gnp.tile([P, 512], f32, name="wrm_ps", tag="wrmps", bufs=1)
        for _ in range(40):
            nc.tensor.matmul(wrm_ps, lhsT=wrm[:, 0:P], rhs=wrm,
                             start=True, stop=True, skip_group_check=True)
        # then keep ticking as x blocks land (each waits on its slice's DMA)
        for s in range(8):
            for ci in range(NCH):
                nc.tensor.matmul(wrm_ps,
                                 lhsT=x_sb[ci][:, s * 512:s * 512 + P],
                                 rhs=x_sb[ci][:, s * 512:(s + 1) * 512],
                                 start=True, stop=True, skip_group_check=True)
        # small constants
        nc.sync.dma_start(out=ind16_sb, in_=i16_d)
        nc.sync.dma_start(out=indT_sb, in_=iT_d)
        nc.vector.memset(onec_sb, 1.0)
        nc.vector.memset(oner_sb, 1.0)
        nc.vector.memset(expb_sb, EXPB)
        nsc_sb, nbs_sb = [], []
        for ci in range(NCH):
            t = gn.tile([P, 1], f32, name=f"nsc{ci}", tag=f"nsc{ci}", bufs=1)
            nc.sync.dma_start(out=t, in_=col_ap(ns_d, ci))
            nsc_sb.append(t)
            t = gn.tile([P, 1], f32, name=f"nbs{ci}", tag=f"nbs{ci}", bufs=1)
            nc.sync.dma_start(out=t, in_=col_ap(nb_d, ci))
            nbs_sb.append(t)

        # per-chunk stats as x chunks land
        m2_all = gn.tile([P, 2 * NCH], f32, name="m2_all", tag="m2", bufs=1)
        for ci in range(NCH):
            stats = gn.tile([P, 8, 6], f32, name=f"st{ci}", tag="st")
            for s in range(8):
                nc.vector.bn_stats(out=stats[:, s, :],
                                   in_=x_sb[ci][:, s * 512:(s + 1) * 512])
            mv = gn.tile([P, 2], f32, name=f"mv{ci}", tag="mv")
            nc.vector.bn_aggr(out=mv, in_=stats)
            nc.vector.tensor_copy(out=m2_all[:, 2 * ci:2 * ci + 1], in_=mv[:, 0:1])
            msq = gn.tile([P, 1], f32, name=f"msq{ci}", tag="msq")
            nc.vector.tensor_mul(out=msq, in0=mv[:, 0:1], in1=mv[:, 0:1])
            nc.vector.tensor_add(out=m2_all[:, 2 * ci + 1:2 * ci + 2],
                                 in0=mv[:, 1:2], in1=msq)

        # k weights now (needed first in phase 1b); q/v/p stream later
        def load_w(srcd, dst2, tagp):
            for ci in range(NCH):
                traw = wraw.tile([P, C], f32, name=f"{tagp}r{ci}", tag="wraw")
                nc.sync.dma_start(out=traw, in_=srcd[ci * P:(ci + 1) * P, :])
                nc.scalar.copy(out=dst2[ci // 2][:, ci % 2, :], in_=traw)
        load_w(wkT_d, wk2, "wk")
        for i in range(NCH):
            nc.sync.dma_start(out=bk_sb[i], in_=col_ap(bk_d, i))
            nc.sync.dma_start(out=bq_sb[i], in_=col_ap(bq_d, i))
        nc.sync.dma_start(out=bp_sb[0], in_=col_ap(bp_d, 0))

        # group aggregation: 32 groups of 16 channels
        g_ps = gnp.tile([8, 2 * NCH], f32, name="g_ps", tag="gps")
        nc.tensor.matmul(g_ps, lhsT=ind16_sb, rhs=m2_all, start=True, stop=True)
        g_sb = gn.tile([8, 2 * NCH], f32, name="g_sb", tag="gsb", bufs=1)
        nc.vector.tensor_copy(out=g_sb, in_=g_ps)
        gv = g_sb.rearrange("p (c two) -> p c two", two=2)
        msq2 = gn.tile([8, NCH], f32, name="msq2", tag="msq2", bufs=1)
        nc.vector.tensor_mul(out=msq2, in0=gv[:, :, 0], in1=gv[:, :, 0])
        var_g = gn.tile([8, NCH], f32, name="var_g", tag="varg", bufs=1)
        nc.vector.tensor_sub(out=var_g, in0=gv[:, :, 1], in1=msq2)
        eps_t = gn.tile([8, 1], f32, name="eps_t", tag="eps", bufs=1)
        nc.vector.memset(eps_t, EPS)
        std_g = gn.tile([8, NCH], f32, name="std_g", tag="stdg", bufs=1)
        nc.scalar.activation(out=std_g, in_=var_g, func=AF.Sqrt,
                             bias=eps_t, scale=1.0)
        rstd_g = gn.tile([8, NCH], f32, name="rstd_g", tag="rstdg", bufs=1)
        nc.vector.reciprocal(out=rstd_g, in_=std_g)
        mr_g = gn.tile([8, NCH], f32, name="mr_g", tag="mrg", bufs=1)
        nc.vector.tensor_mul(out=mr_g, in0=gv[:, :, 0], in1=rstd_g)
        rb2 = gn.tile([8, 2 * NCH], f32, name="rb2", tag="rb2", bufs=1)
        rv = rb2.rearrange("p (c two) -> p c two", two=2)
        nc.vector.tensor_copy(out=rv[:, :, 0], in_=rstd_g)
        nc.vector.tensor_copy(out=rv[:, :, 1], in_=mr_g)

        for ci in range(NCH):
            ab_ps = gnp.tile([P, 2], f32, name=f"ab{ci}", tag="gps")
            nc.tensor.matmul(ab_ps, lhsT=indT_sb, rhs=rb2[:, 2 * ci:2 * ci + 2],
                             start=True, stop=True)
            A_t = gn.tile([P, 1], f32, name=f"A{ci}", tag="A")
            nc.vector.tensor_mul(out=A_t, in0=ab_ps[:, 0:1], in1=nsc_sb[ci])
            t0 = gn.tile([P, 1], f32, name=f"t0{ci}", tag="t0")
            nc.vector.tensor_mul(out=t0, in0=ab_ps[:, 1:2], in1=nsc_sb[ci])
            B_t = gn.tile([P, 1], f32, name=f"B{ci}", tag="Bt")
            nc.vector.tensor_sub(out=B_t, in0=nbs_sb[ci], in1=t0)
            # h = x*A + B, split across DVE and ScalarE to halve head latency
            if ci < 2:
                nc.vector.tensor_scalar(out=h2[ci // 2][:, ci % 2, :],
                                        in0=x_sb[ci],
                                        scalar1=A_t, scalar2=B_t,
                                        op0=OP.mult, op1=OP.add)
            else:
                nc.scalar.activation(out=h2[ci // 2][:, ci % 2, :],
                                     in_=x_sb[ci], func=AF.Identity,
                                     bias=B_t, scale=A_t)

        for i in range(1, NCH):
            nc.sync.dma_start(out=bp_sb[i], in_=col_ap(bp_d, i))

    # q/k/vT fully SBUF-resident in fp8; allocated only now so phase 1a
    # could use this space for x
    vt_pool = ctx.enter_context(tc.tile_pool(name="vtres", bufs=1))
    vt2 = [vt_pool.tile([P, 2, C], f8, name=f"vt{e}", tag=f"vt{e}")
           for e in range(NE2)]
    k_pool = ctx.enter_context(tc.tile_pool(name="kres", bufs=1))
    k2 = [k_pool.tile([P, 2, HW], f8, name=f"k{i}", tag=f"k{i}")
          for i in range(NC2)]
    q_pool = ctx.enter_context(tc.tile_pool(name="qres", bufs=1))
    q2 = [q_pool.tile([P, 2, HW], f8, name=f"q{i}", tag=f"q{i}")
          for i in range(NC2)]

    # ================ phase 1b: k, q, vT -> resident sbuf (fp8 DR) ========
    # Paired pixel blocks ([P,2,512] PSUM tiles) halve the PSUM->SBUF
    # consumer instruction count; consumers are split across ScalarE (k)
    # and VectorE (q) with vT copies alternating between the two.
    with ExitStack() as s2:
        pp1 = s2.enter_context(tc.tile_pool(name="pp1", bufs=4, space="PSUM"))
        wraw2 = s2.enter_context(tc.tile_pool(name="wraw2", bufs=4))

        def load_w2(srcd, dst2, tagp, eng):
            for ci in range(NCH):
                traw = wraw2.tile([P, C], f32, name=f"{tagp}r{ci}", tag="wraw2")
                nc.sync.dma_start(out=traw, in_=srcd[ci * P:(ci + 1) * P, :])
                if eng == "sc":
                    nc.scalar.copy(out=dst2[ci // 2][:, ci % 2, :], in_=traw)
                else:
                    nc.vector.tensor_copy(out=dst2[ci // 2][:, ci % 2, :],
                                          in_=traw)

        for nbp in range(NB // 2):
            for m in range(NCH):
                t = pp1.tile([P, 2, 512], f32, name="kps", tag="mm")
                for i in range(2):
                    nb = 2 * nbp + i
                    for c2 in range(NC2):
                        nc.tensor.matmul(t[:, i, :],
                                         lhsT=wk2[c2][:, :, m * P:(m + 1) * P],
                                         rhs=h2[c2][:, :, nb * 512:(nb + 1) * 512],
                                         start=(c2 == 0), stop=(c2 == NC2 - 1),
                                         perf_mode=DR)
                nc.scalar.activation(
                    out=k2[m // 2][:, m % 2, nbp * 1024:(nbp + 1) * 1024],
                    in_=t, func=AF.Identity, bias=bk_sb[m], scale=1.0)
            if nbp == 0:
                # q weights converted on the (currently idle) DVE while
                # the k matmuls stream
                load_w2(wqT_d, wq2, "wq", "ve")
        for nbp in range(NB // 2):
            for m in range(NCH):
                t = pp1.tile([P, 2, 512], f32, name="qps", tag="mm")
                for i in range(2):
                    nb = 2 * nbp + i
                    for c2 in range(NC2):
                        nc.tensor.matmul(t[:, i, :],
                                         lhsT=wq2[c2][:, :, m * P:(m + 1) * P],
                                         rhs=h2[c2][:, :, nb * 512:(nb + 1) * 512],
                                         start=(c2 == 0), stop=(c2 == NC2 - 1),
                                         perf_mode=DR)
                nc.vector.tensor_scalar_add(
                    out=q2[m // 2][:, m % 2, nbp * 1024:(nbp + 1) * 1024],
                    in0=t, scalar1=bq_sb[m])
            if nbp == 0:
                # v weights on ScalarE (its k-act backlog is short by now)
                load_w2(wvT_d, wv2, "wv", "sc")
        # p weights during the vT stretch
        load_w2(wpT_d, wpT_sb, "wp", "sc")
        for e2 in range(NE2):
            t = pp1.tile([P, 2, 512], f32, name="vps", tag="mm")
            for j in range(2):
                e = 2 * e2 + j
                for c2 in range(NC2):
                    nc.tensor.matmul(t[:, j, :],
                                     lhsT=h2[c2][:, :, e * P:(e + 1) * P],
                                     rhs=wv2[c2],
                                     start=(c2 == 0), stop=(c2 == NC2 - 1),
                                     perf_mode=DR)
            # bv is folded into bp' on host, so vT is a plain copy;
            # alternate engines to balance load
            if e2 % 2 == 0:
                nc.scalar.copy(out=vt2[e2], in_=t)
            else:
                nc.vector.tensor_copy(out=vt2[e2], in_=t)

    h_pool.release()
    w_pool.release()

    # ================ phase 2: attention + proj ================
    with ExitStack() as s3:
        qxp = s3.enter_context(tc.tile_pool(name="qxp", bufs=8))
        ptp = s3.enter_context(tc.tile_pool(name="ptp", bufs=3))
        smp = s3.enter_context(tc.tile_pool(name="smp", bufs=2))
        osp = s3.enter_context(tc.tile_pool(name="osp", bufs=2))
        fnp = s3.enter_context(tc.tile_pool(name="fnp", bufs=2))
        pvp = s3.enter_context(tc.tile_pool(name="pvp", bufs=4, space="PSUM"))
        ssp = s3.enter_context(tc.tile_pool(name="ssp", bufs=3, space="PSUM"))
        smps = s3.enter_context(tc.tile_pool(name="smps", bufs=1, space="PSUM"))

        def make_stages(qb, osb2, dn_sb, xcur, last=False):
            hold = {}

            def st_rb():
                rb_ps = smps.tile([P, QB], f32, name="rb_ps", tag="sm")
                nc.tensor.matmul(rb_ps, lhsT=oner_sb, rhs=dn_sb,
                                 start=True, stop=True)
                rb_sb = smp.tile([P, QB], f32, name="rb_sb", tag="rbsb", bufs=2)
                nc.vector.reciprocal_approx_fast(out=rb_sb, in_=rb_ps)
                hold["rb"] = rb_sb

            def mk_pj(oc):
                def st_pj():
                    # the drain after the final query block has no S-loop to
                    # interleave with; use the freed PV banks so the four
                    # projections pipeline instead of serializing on one bank
                    pool, tag = (pvp, "pv") if last else (smps, "sm")
                    pj_ps = pool.tile([P, QB], f32, name="pj_ps", tag=tag)
                    for c2 in range(NC2):
                        nc.tensor.matmul(pj_ps,
                                         lhsT=wpT_sb[c2][:, :, oc * P:(oc + 1) * P],
                                         rhs=osb2[c2],
                                         start=(c2 == 0), stop=(c2 == NC2 - 1),
                                         perf_mode=DR)
                    t1 = fnp.tile([P, QB], f32, name="t1", tag="t1", bufs=2)
                    nc.vector.tensor_mul(out=t1, in0=pj_ps, in1=hold["rb"])
                    fin = fnp.tile([P, QB], f32, name="fin", tag="fin", bufs=2)
                    nc.vector.scalar_tensor_tensor(out=fin, in0=t1,
                                                   scalar=bp_sb[oc],
                                                   in1=xcur[oc],
                                                   op0=OP.add, op1=OP.add)
                    nc.sync.dma_start(out=out_d[oc * P:(oc + 1) * P,
                                                qb * QB:(qb + 1) * QB], in_=fin)
                return st_pj

            return [st_rb, mk_pj(0), mk_pj(1), mk_pj(2), mk_pj(3)]

        stages = []
        for qb in range(NQB):
            xcur = []
            for ci in range(NCH):
                t = qxp.tile([P, QB], f32, name=f"xb{ci}", tag="xblk")
                nc.sync.dma_start(out=t, in_=x_d[ci * P:(ci + 1) * P,
                                                 qb * QB:(qb + 1) * QB])
                xcur.append(t)
            acc2 = smp.tile([P, 2, QB], bf16, name="acc2", tag="acc")
            pvs = None
            for e2 in range(NE2):
                pt2 = ptp.tile([P, 2, QB], f8, name="pt", tag="pt")
                for j in range(2):
                    e = 2 * e2 + j
                    s_ps = ssp.tile([P, QB], f32, name="s_ps", tag="s")
                    for c2 in range(NC2):
                        nc.tensor.matmul(s_ps,
                                         lhsT=k2[c2][:, :, e * P:(e + 1) * P],
                                         rhs=q2[c2][:, :, qb * QB:(qb + 1) * QB],
                                         start=(c2 == 0), stop=(c2 == NC2 - 1),
                                         perf_mode=DR)
                    nc.scalar.activation(out=pt2[:, j, :], in_=s_ps, func=AF.Exp,
                                         bias=expb_sb, scale=float(SCALE))
                # paired accumulate (both j lanes in one DVE op)
                if e2 == 0:
                    nc.vector.tensor_copy(out=acc2, in_=pt2)
                else:
                    nc.vector.tensor_add(out=acc2, in0=acc2, in1=pt2)
                # previous qb's projection work, interleaved so the PE
                # never waits on the recip chain
                if stages:
                    stages.pop(0)()
                if e2 == 0:
                    pvs = [pvp.tile([P, QB], f32, name=f"pv{co}", tag="pv")
                           for co in range(NCH)]
                for co in range(NCH):
                    nc.tensor.matmul(pvs[co],
                                     lhsT=vt2[e2][:, :, co * P:(co + 1) * P],
                                     rhs=pt2,
                                     start=(e2 == 0), stop=(e2 == NE2 - 1),
                                     perf_mode=DR)
            # fold the two acc lanes, then denominator matmul
            accf = smp.tile([P, QB], bf16, name="accf", tag="accf", bufs=2)
            nc.vector.tensor_add(out=accf, in0=acc2[:, 0, :], in1=acc2[:, 1, :])
            dnrb = smps.tile([P, QB], f32, name="dnrb", tag="sm")
            nc.tensor.matmul(dnrb[0:1, :], lhsT=onec_sb, rhs=accf,
                             start=True, stop=True)
            dn_sb = smp.tile([1, QB], bf16, name="dn_sb", tag="dnsb", bufs=2)
            nc.scalar.copy(out=dn_sb, in_=dnrb[0:1, :])
            osb2 = [osp.tile([P, 2, QB], f8, name=f"osb{c2}", tag=f"osb{c2}",
                             bufs=2) for c2 in range(NC2)]
            for co in range(NCH):
                if co < 2:
                    nc.scalar.copy(out=osb2[co // 2][:, co % 2, :], in_=pvs[co])
                else:
                    nc.vector.tensor_copy(out=osb2[co // 2][:, co % 2, :],
                                          in_=pvs[co])
            stages = make_stages(qb, osb2, dn_sb, xcur, last=(qb == NQB - 1))
        while stages:
            stages.pop(0)()


def build_nc():
    import concourse.bacc as bacc
    import concourse.tile as tile
    from contextlib import ExitStack

    nc = bacc.Bacc("TRN2", target_bir_lowering=False, debug=False)
    with tile.TileContext(nc) as tc:
        with ExitStack() as ctx:
            _emit(nc, tc, ctx)
    nc.finalize()
    return nc


def host_constants():
    ind16 = np.zeros((P, 8), np.float32)
    for p in range(P):
        ind16[p, p // 16] = 1.0 / 16.0
    indT = np.zeros((8, P), np.float32)
    for p in range(P):
        indT[p // 16, p] = 1.0
    return ind16, indT


def make_in_maps(inputs):
    x = np.asarray(inputs["x"], np.float32)
    ind16, indT = host_constants()
    wp = np.asarray(inputs["wp"], np.float32)
    bv = np.asarray(inputs["bv"], np.float32)
    # bv enters the output only as wp @ (bv * sum(attn)) = wp @ bv per query
    # (attention rows sum to 1), so fold it into the projection bias.
    bp_prime = np.asarray(inputs["bp"], np.float32) + wp @ bv
    shared = {
        "wqT": np.ascontiguousarray(np.asarray(inputs["wq"], np.float32).T),
        "wkT": np.ascontiguousarray(np.asarray(inputs["wk"], np.float32).T),
        "wvT": np.ascontiguousarray(np.asarray(inputs["wv"], np.float32).T),
        "wpT": np.ascontiguousarray(wp.T),
        "bq": np.asarray(inputs["bq"], np.float32),
        "bk": np.asarray(inputs["bk"], np.float32),
        "bp": bp_prime,
        "nscale": np.asarray(inputs["norm_scale"], np.float32),
        "nbias": np.asarray(inputs["norm_bias"], np.float32),
        "ind16": ind16, "indT": indT,
    }
    return [dict(shared, x=np.ascontiguousarray(x[i].reshape(C, HW)))
            for i in range(B)]


def kernel(**inputs):
    from concourse.bass_utils import run_bass_kernel_spmd

    if "nc" not in _CACHE:
        _CACHE["nc"] = build_nc()
    nc = _CACHE["nc"]
    in_maps = make_in_maps(inputs)
    res = run_bass_kernel_spmd(nc, in_maps, list(range(B)))
    out = np.stack([res.results[i]["out"] for i in range(B)])
    return out.reshape(B, C, 64, 64)


# revision 26
# speedup vs baseline: 1.0695x; 1.0695x over previous
"""AttnBlock2D Trainium2 kernel (8-core data-parallel over batch).

Per core: one batch element. x:[512, 4096] (c, h*w).
  h = GroupNorm32(x) * scale + bias
  q = wq@h, k = wk@h, v = wv@h          (1x1 conv == matmul over channels)
  attn = softmax(q^T k / sqrt(512));  out = v @ attn^T
  y = x + wp@out + bp'                  (bp' = bp + wp@bv, folded on host)

On-chip layout: S^T orientation ([keys on partitions, queries on free]) so the
softmax denominator is a partition-dim reduction done with a ones-matmul; the
PV contraction then needs no transposes at all (vT is produced directly by
swapping matmul operands). Softmax skips max-subtraction: scores are ~N(0,1)
by construction; exp() gets a -2 bias so fp8e4m3 outputs can't overflow (the
factor cancels between numerator and denominator).

All big matmuls run fp8e4m3 with perf_mode=DoubleRow: operands are stored as
[128, 2, n] (contraction = 128 partitions x 2 interleaved k-tiles), which
halves instruction count and PE streaming time vs bf16. PSUM stays fp32.
Attention-path quantization error is diluted ~40x in the output norm by the
residual, so rel-err stays ~5e-3, under the 2e-2 gate.

Softmax normalization is applied LATE (after the output projection):
y = x + (wp@U)*r + bp' with U the unnormalized PV output and r = 1/denominator
per query. This lets U leave PSUM as plain copies right after the last PV
(freeing the PV accumulator banks for the next query block immediately), and
the projection matmuls + normalize are emitted as pending stages interleaved
into the next query block's S-loop so the PE never waits on the recip chain.
"""
import os
import numpy as np

P = 128
C = 512
NCH = C // P              # 4 channel chunks
NC2 = NCH // 2            # 2 channel pair-chunks (DoubleRow)
HW = 4096                 # 64*64 pixels
QB = 512                  # query block (PSUM-bank limited)
NQB = HW // QB            # 8
NE = HW // P              # 32 key chunks
NE2 = NE // 2             # 16 key pair-chunks
NB = HW // 512            # 8 pixel blocks for projections
EPS = 1e-5
SCALE = 1.0 / np.sqrt(C)
# exp bias: scores are ~N(0,1), so exp(s - 3.39) keeps the fp8 probabilities
# and (more importantly) the unnormalized PV output U = sum(pt*v) safely
# below e4m3's 240 max while staying above the subnormal floor for all but
# a ~4% tail. The constant cancels between softmax numerator & denominator.
EXPB = -2.0 - float(np.log(4.0))
B = 8                     # batch / cores

_CACHE = {}


def _emit(nc, tc, ctx):
    import concourse.bass as bass
    from concourse import mybir
    from contextlib import ExitStack

    f32 = mybir.dt.float32
    bf16 = mybir.dt.bfloat16
    f8 = mybir.dt.float8e4
    AF = mybir.ActivationFunctionType
    OP = mybir.AluOpType
    DR = mybir.MatmulPerfMode.DoubleRow

    # ---------------- I/O ----------------
    x_d = nc.declare_dram_parameter("x", [C, HW], f32, isOutput=False).ap()
    wqT_d = nc.declare_dram_parameter("wqT", [C, C], f32, isOutput=False).ap()
    wkT_d = nc.declare_dram_parameter("wkT", [C, C], f32, isOutput=False).ap()
    wvT_d = nc.declare_dram_parameter("wvT", [C, C], f32, isOutput=False).ap()
    wpT_d = nc.declare_dram_parameter("wpT", [C, C], f32, isOutput=False).ap()
    bq_d = nc.declare_dram_parameter("bq", [C], f32, isOutput=False).ap()
    bk_d = nc.declare_dram_parameter("bk", [C], f32, isOutput=False).ap()
    bp_d = nc.declare_dram_parameter("bp", [C], f32, isOutput=False).ap()
    ns_d = nc.declare_dram_parameter("nscale", [C], f32, isOutput=False).ap()
    nb_d = nc.declare_dram_parameter("nbias", [C], f32, isOutput=False).ap()
    i16_d = nc.declare_dram_parameter("ind16", [P, 8], f32, isOutput=False).ap()
    iT_d = nc.declare_dram_parameter("indT", [8, P], f32, isOutput=False).ap()
    out_d = nc.declare_dram_parameter("out", [C, HW], f32, isOutput=True).ap()

    def col_ap(src, ci):
        # [128] slice of a [512] DRAM vector viewed as [128, 1]
        return bass.AP(tensor=src.tensor, offset=ci * P, ap=[[1, P], [0, 1]])

    # ---------------- persistent pools (allocated before phase-1a scope) ----
    cst = ctx.enter_context(tc.tile_pool(name="cst", bufs=1))
    wpT_sb = [cst.tile([P, 2, C], f8, name=f"wpT{i}", tag=f"wpT{i}")
              for i in range(NC2)]
    ind16_sb = cst.tile([P, 8], f32, name="ind16", tag="ind16")
    indT_sb = cst.tile([8, P], f32, name="indT", tag="indT")
    onec_sb = cst.tile([P, 1], bf16, name="onec", tag="onec")
    oner_sb = cst.tile([1, P], bf16, name="oner", tag="oner")
    bp_sb = [cst.tile([P, 1], f32, name=f"bp{i}", tag=f"bp{i}") for i in range(NCH)]
    expb_sb = cst.tile([P, 1], f32, name="expb", tag="expb")

    w_pool = tc.alloc_tile_pool(name="wqkv", bufs=1, side="right")
    wq2 = [w_pool.tile([P, 2, C], f8, name=f"wq2{i}", tag=f"wq2{i}") for i in range(NC2)]
    wk2 = [w_pool.tile([P, 2, C], f8, name=f"wk2{i}", tag=f"wk2{i}") for i in range(NC2)]
    wv2 = [w_pool.tile([P, 2, C], f8, name=f"wv2{i}", tag=f"wv2{i}") for i in range(NC2)]
    bq_sb = [w_pool.tile([P, 1], f32, name=f"bq{i}", tag=f"bq{i}") for i in range(NCH)]
    bk_sb = [w_pool.tile([P, 1], f32, name=f"bk{i}", tag=f"bk{i}") for i in range(NCH)]

    h_pool = tc.alloc_tile_pool(name="hres", bufs=1, side="right")
    h2 = [h_pool.tile([P, 2, HW], f8, name=f"h{i}", tag=f"h{i}") for i in range(NC2)]

    # ================ phase 1a: groupnorm (x DMA emitted first) ============
    with ExitStack() as s1:
        xp = s1.enter_context(tc.tile_pool(name="xp", bufs=1))
        wraw = s1.enter_context(tc.tile_pool(name="wraw", bufs=4))
        gn = s1.enter_context(tc.tile_pool(name="gn", bufs=2))
        gnp = s1.enter_context(tc.tile_pool(name="gnp", bufs=2, space="PSUM"))

        # x first: it gates the whole head (stats -> h -> qkv)
        x_sb = []
        for ci in range(NCH):
            t = xp.tile([P, HW], f32, name=f"x{ci}", tag=f"x{ci}")
            nc.sync.dma_start(out=t, in_=x_d[ci * P:(ci + 1) * P, :])
            x_sb.append(t)
        # small constants
        nc.sync.dma_start(out=ind16_sb, in_=i16_d)
        nc.sync.dma_start(out=indT_sb, in_=iT_d)
        nc.vector.memset(onec_sb, 1.0)
        nc.vector.memset(oner_sb, 1.0)
        nc.vector.memset(expb_sb, EXPB)
        nsc_sb, nbs_sb = [], []
        for ci in range(NCH):
            t = gn.tile([P, 1], f32, name=f"nsc{ci}", tag=f"nsc{ci}", bufs=1)
            nc.sync.dma_start(out=t, in_=col_ap(ns_d, ci))
            nsc_sb.append(t)
            t = gn.tile([P, 1], f32, name=f"nbs{ci}", tag=f"nbs{ci}", bufs=1)
            nc.sync.dma_start(out=t, in_=col_ap(nb_d, ci))
            nbs_sb.append(t)

        # per-chunk stats as x chunks land
        m2_all = gn.tile([P, 2 * NCH], f32, name="m2_all", tag="m2", bufs=1)
        for ci in range(NCH):
            stats = gn.tile([P, 8, 6], f32, name=f"st{ci}", tag="st")
            for s in range(8):
                nc.vector.bn_stats(out=stats[:, s, :],
                                   in_=x_sb[ci][:, s * 512:(s + 1) * 512])
            mv = gn.tile([P, 2], f32, name=f"mv{ci}", tag="mv")
            nc.vector.bn_aggr(out=mv, in_=stats)
            nc.vector.tensor_copy(out=m2_all[:, 2 * ci:2 * ci + 1], in_=mv[:, 0:1])
            msq = gn.tile([P, 1], f32, name=f"msq{ci}", tag="msq")
            nc.vector.tensor_mul(out=msq, in0=mv[:, 0:1], in1=mv[:, 0:1])
            nc.vector.tensor_add(out=m2_all[:, 2 * ci + 1:2 * ci + 2],
                                 in0=mv[:, 1:2], in1=msq)

        # k weights now (needed first in phase 1b); q/v/p stream later
        def load_w(srcd, dst2, tagp):
            for ci in range(NCH):
                traw = wraw.tile([P, C], f32, name=f"{tagp}r{ci}", tag="wraw")
                nc.sync.dma_start(out=traw, in_=srcd[ci * P:(ci + 1) * P, :])
                nc.scalar.copy(out=dst2[ci // 2][:, ci % 2, :], in_=traw)
        load_w(wkT_d, wk2, "wk")
        for i in range(NCH):
            nc.sync.dma_start(out=bk_sb[i], in_=col_ap(bk_d, i))
            nc.sync.dma_start(out=bq_sb[i], in_=col_ap(bq_d, i))
        nc.sync.dma_start(out=bp_sb[0], in_=col_ap(bp_d, 0))

        # group aggregation: 32 groups of 16 channels
        g_ps = gnp.tile([8, 2 * NCH], f32, name="g_ps", tag="gps")
        nc.tensor.matmul(g_ps, lhsT=ind16_sb, rhs=m2_all, start=True, stop=True)
        g_sb = gn.tile([8, 2 * NCH], f32, name="g_sb", tag="gsb", bufs=1)
        nc.vector.tensor_copy(out=g_sb, in_=g_ps)
        gv = g_sb.rearrange("p (c two) -> p c two", two=2)
        msq2 = gn.tile([8, NCH], f32, name="msq2", tag="msq2", bufs=1)
        nc.vector.tensor_mul(out=msq2, in0=gv[:, :, 0], in1=gv[:, :, 0])
        var_g = gn.tile([8, NCH], f32, name="var_g", tag="varg", bufs=1)
        nc.vector.tensor_sub(out=var_g, in0=gv[:, :, 1], in1=msq2)
        eps_t = gn.tile([8, 1], f32, name="eps_t", tag="eps", bufs=1)
        nc.vector.memset(eps_t, EPS)
        std_g = gn.tile([8, NCH], f32, name="std_g", tag="stdg", bufs=1)
        nc.scalar.activation(out=std_g, in_=var_g, func=AF.Sqrt,
                             bias=eps_t, scale=1.0)
        rstd_g = gn.tile([8, NCH], f32, name="rstd_g", tag="rstdg", bufs=1)
        nc.vector.reciprocal(out=rstd_g, in_=std_g)
        mr_g = gn.tile([8, NCH], f32, name="mr_g", tag="mrg", bufs=1)
        nc.vector.tensor_mul(out=mr_g, in0=gv[:, :, 0], in1=rstd_g)
        rb2 = gn.tile([8, 2 * NCH], f32, name="rb2", tag="rb2", bufs=1)
        rv = rb2.rearrange("p (c two) -> p c two", two=2)
        nc.vector.tensor_copy(out=rv[:, :, 0], in_=rstd_g)
        nc.vector.tensor_copy(out=rv[:, :, 1], in_=mr_g)

        for ci in range(NCH):
            ab_ps = gnp.tile([P, 2], f32, name=f"ab{ci}", tag="gps")
            nc.tensor.matmul(ab_ps, lhsT=indT_sb, rhs=rb2[:, 2 * ci:2 * ci + 2],
                             start=True, stop=True)
            A_t = gn.tile([P, 1], f32, name=f"A{ci}", tag="A")
            nc.vector.tensor_mul(out=A_t, in0=ab_ps[:, 0:1], in1=nsc_sb[ci])
            t0 = gn.tile([P, 1], f32, name=f"t0{ci}", tag="t0")
            nc.vector.tensor_mul(out=t0, in0=ab_ps[:, 1:2], in1=nsc_sb[ci])
            B_t = gn.tile([P, 1], f32, name=f"B{ci}", tag="Bt")
            nc.vector.tensor_sub(out=B_t, in0=nbs_sb[ci], in1=t0)
            # h = x*A + B, split across DVE and ScalarE to halve head latency
            if ci < 2:
                nc.vector.tensor_scalar(out=h2[ci // 2][:, ci % 2, :],
                                        in0=x_sb[ci],
                                        scalar1=A_t, scalar2=B_t,
                                        op0=OP.mult, op1=OP.add)
            else:
                nc.scalar.activation(out=h2[ci // 2][:, ci % 2, :],
                                     in_=x_sb[ci], func=AF.Identity,
                                     bias=B_t, scale=A_t)

        # remaining weights: emitted after h (execute during phase 1b)
        load_w(wqT_d, wq2, "wq")
        load_w(wvT_d, wv2, "wv")
        for ci in range(NCH):
            traw = wraw.tile([P, C], f32, name=f"wpr{ci}", tag="wraw")
            nc.sync.dma_start(out=traw, in_=wpT_d[ci * P:(ci + 1) * P, :])
            nc.scalar.copy(out=wpT_sb[ci // 2][:, ci % 2, :], in_=traw)
        for i in range(1, NCH):
            nc.sync.dma_start(out=bp_sb[i], in_=col_ap(bp_d, i))

    # q/k/vT fully SBUF-resident in fp8; allocated only now so phase 1a
    # could use this space for x
    vt_pool = ctx.enter_context(tc.tile_pool(name="vtres", bufs=1))
    vt2 = [vt_pool.tile([P, 2, C], f8, name=f"vt{e}", tag=f"vt{e}")
           for e in range(NE2)]
    k_pool = ctx.enter_context(tc.tile_pool(name="kres", bufs=1))
    k2 = [k_pool.tile([P, 2, HW], f8, name=f"k{i}", tag=f"k{i}")
          for i in range(NC2)]
    q_pool = ctx.enter_context(tc.tile_pool(name="qres", bufs=1))
    q2 = [q_pool.tile([P, 2, HW], f8, name=f"q{i}", tag=f"q{i}")
          for i in range(NC2)]

    # ================ phase 1b: k, q, vT -> resident sbuf (fp8 DR) ========
    # Paired pixel blocks ([P,2,512] PSUM tiles) halve the PSUM->SBUF
    # consumer instruction count; consumers are split across ScalarE (k)
    # and VectorE (q) with vT copies alternating between the two.
    with ExitStack() as s2:
        pp1 = s2.enter_context(tc.tile_pool(name="pp1", bufs=4, space="PSUM"))
        for nbp in range(NB // 2):
            for m in range(NCH):
                t = pp1.tile([P, 2, 512], f32, name="kps", tag="mm")
                for i in range(2):
                    nb = 2 * nbp + i
                    for c2 in range(NC2):
                        nc.tensor.matmul(t[:, i, :],
                                         lhsT=wk2[c2][:, :, m * P:(m + 1) * P],
                                         rhs=h2[c2][:, :, nb * 512:(nb + 1) * 512],
                                         start=(c2 == 0), stop=(c2 == NC2 - 1),
                                         perf_mode=DR)
                nc.scalar.activation(
                    out=k2[m // 2][:, m % 2, nbp * 1024:(nbp + 1) * 1024],
                    in_=t, func=AF.Identity, bias=bk_sb[m], scale=1.0)
        for nbp in range(NB // 2):
            for m in range(NCH):
                t = pp1.tile([P, 2, 512], f32, name="qps", tag="mm")
                for i in range(2):
                    nb = 2 * nbp + i
                    for c2 in range(NC2):
                        nc.tensor.matmul(t[:, i, :],
                                         lhsT=wq2[c2][:, :, m * P:(m + 1) * P],
                                         rhs=h2[c2][:, :, nb * 512:(nb + 1) * 512],
                                         start=(c2 == 0), stop=(c2 == NC2 - 1),
                                         perf_mode=DR)
                nc.vector.tensor_scalar_add(
                    out=q2[m // 2][:, m % 2, nbp * 1024:(nbp + 1) * 1024],
                    in0=t, scalar1=bq_sb[m])
        for e2 in range(NE2):
            t = pp1.tile([P, 2, 512], f32, name="vps", tag="mm")
            for j in range(2):
                e = 2 * e2 + j
                for c2 in range(NC2):
                    nc.tensor.matmul(t[:, j, :],
                                     lhsT=h2[c2][:, :, e * P:(e + 1) * P],
                                     rhs=wv2[c2],
                                     start=(c2 == 0), stop=(c2 == NC2 - 1),
                                     perf_mode=DR)
            # bv is folded into bp' on host, so vT is a plain copy;
            # alternate engines to balance load
            if e2 % 2 == 0:
                nc.scalar.copy(out=vt2[e2], in_=t)
            else:
                nc.vector.tensor_copy(out=vt2[e2], in_=t)

    h_pool.release()
    w_pool.release()

    # ================ phase 2: attention + proj ================
    with ExitStack() as s3:
        qxp = s3.enter_context(tc.tile_pool(name="qxp", bufs=8))
        ptp = s3.enter_context(tc.tile_pool(name="ptp", bufs=3))
        smp = s3.enter_context(tc.tile_pool(name="smp", bufs=2))
        osp = s3.enter_context(tc.tile_pool(name="osp", bufs=2))
        fnp = s3.enter_context(tc.tile_pool(name="fnp", bufs=2))
        pvp = s3.enter_context(tc.tile_pool(name="pvp", bufs=4, space="PSUM"))
        ssp = s3.enter_context(tc.tile_pool(name="ssp", bufs=3, space="PSUM"))
        smps = s3.enter_context(tc.tile_pool(name="smps", bufs=1, space="PSUM"))

        def make_stages(qb, osb2, dn_sb, xcur, last=False):
            hold = {}

            def st_rb():
                rb_ps = smps.tile([P, QB], f32, name="rb_ps", tag="sm")
                nc.tensor.matmul(rb_ps, lhsT=oner_sb, rhs=dn_sb,
                                 start=True, stop=True)
                rb_sb = smp.tile([P, QB], f32, name="rb_sb", tag="rbsb", bufs=2)
                nc.vector.reciprocal_approx_fast(out=rb_sb, in_=rb_ps)
                hold["rb"] = rb_sb

            def mk_pj(oc):
                def st_pj():
                    # the drain after the final query block has no S-loop to
                    # interleave with; use the freed PV banks so the four
                    # projections pipeline instead of serializing on one bank
                    pool, tag = (pvp, "pv") if last else (smps, "sm")
                    pj_ps = pool.tile([P, QB], f32, name="pj_ps", tag=tag)
                    for c2 in range(NC2):
                        nc.tensor.matmul(pj_ps,
                                         lhsT=wpT_sb[c2][:, :, oc * P:(oc + 1) * P],
                                         rhs=osb2[c2],
                                         start=(c2 == 0), stop=(c2 == NC2 - 1),
                                         perf_mode=DR)
                    t1 = fnp.tile([P, QB], f32, name="t1", tag="t1", bufs=2)
                    nc.vector.tensor_mul(out=t1, in0=pj_ps, in1=hold["rb"])
                    fin = fnp.tile([P, QB], f32, name="fin", tag="fin", bufs=2)
                    nc.vector.scalar_tensor_tensor(out=fin, in0=t1,
                                                   scalar=bp_sb[oc],
                                                   in1=xcur[oc],
                                                   op0=OP.add, op1=OP.add)
                    nc.sync.dma_start(out=out_d[oc * P:(oc + 1) * P,
                                                qb * QB:(qb + 1) * QB], in_=fin)
                return st_pj

            return [st_rb, mk_pj(0), mk_pj(1), mk_pj(2), mk_pj(3)]

        stages = []
        for qb in range(NQB):
            xcur = []
            for ci in range(NCH):
                t = qxp.tile([P, QB], f32, name=f"xb{ci}", tag="xblk")
                nc.sync.dma_start(out=t, in_=x_d[ci * P:(ci + 1) * P,
                                                 qb * QB:(qb + 1) * QB])
                xcur.append(t)
            acc2 = smp.tile([P, 2, QB], bf16, name="acc2", tag="acc")
            pvs = None
            for e2 in range(NE2):
                pt2 = ptp.tile([P, 2, QB], f8, name="pt", tag="pt")
                for j in range(2):
                    e = 2 * e2 + j
                    s_ps = ssp.tile([P, QB], f32, name="s_ps", tag="s")
                    for c2 in range(NC2):
                        nc.tensor.matmul(s_ps,
                                         lhsT=k2[c2][:, :, e * P:(e + 1) * P],
                                         rhs=q2[c2][:, :, qb * QB:(qb + 1) * QB],
                                         start=(c2 == 0), stop=(c2 == NC2 - 1),
                                         perf_mode=DR)
                    nc.scalar.activation(out=pt2[:, j, :], in_=s_ps, func=AF.Exp,
                                         bias=expb_sb, scale=float(SCALE))
                # paired accumulate (both j lanes in one DVE op)
                if e2 == 0:
                    nc.vector.tensor_copy(out=acc2, in_=pt2)
                else:
                    nc.vector.tensor_add(out=acc2, in0=acc2, in1=pt2)
                # previous qb's projection work, interleaved so the PE
                # never waits on the recip chain
                if stages:
                    stages.pop(0)()
                if e2 == 0:
                    pvs = [pvp.tile([P, QB], f32, name=f"pv{co}", tag="pv")
                           for co in range(NCH)]
                for co in range(NCH):
                    nc.tensor.matmul(pvs[co],
                                     lhsT=vt2[e2][:, :, co * P:(co + 1) * P],
                                     rhs=pt2,
                                     start=(e2 == 0), stop=(e2 == NE2 - 1),
                                     perf_mode=DR)
            # fold the two acc lanes, then denominator matmul
            accf = smp.tile([P, QB], bf16, name="accf", tag="accf", bufs=2)
            nc.vector.tensor_add(out=accf, in0=acc2[:, 0, :], in1=acc2[:, 1, :])
            dnrb = smps.tile([P, QB], f32, name="dnrb", tag="sm")
            nc.tensor.matmul(dnrb[0:1, :], lhsT=onec_sb, rhs=accf,
                             start=True, stop=True)
            dn_sb = smp.tile([1, QB], bf16, name="dn_sb", tag="dnsb", bufs=2)
            nc.scalar.copy(out=dn_sb, in_=dnrb[0:1, :])
            osb2 = [osp.tile([P, 2, QB], f8, name=f"osb{c2}", tag=f"osb{c2}",
                             bufs=2) for c2 in range(NC2)]
            for co in range(NCH):
                if co < 2:
                    nc.scalar.copy(out=osb2[co // 2][:, co % 2, :], in_=pvs[co])
                else:
                    nc.vector.tensor_copy(out=osb2[co // 2][:, co % 2, :],
                                          in_=pvs[co])
            stages = make_stages(qb, osb2, dn_sb, xcur, last=(qb == NQB - 1))
        while stages:
            stages.pop(0)()


def build_nc():
    import concourse.bacc as bacc
    import concourse.tile as tile
    from contextlib import ExitStack

    nc = bacc.Bacc("TRN2", target_bir_lowering=False, debug=False)
    with tile.TileContext(nc) as tc:
        with ExitStack() as ctx:
            _emit(nc, tc, ctx)
    nc.finalize()
    return nc


def host_constants():
    ind16 = np.zeros((P, 8), np.float32)
    for p in range(P):
        ind16[p, p // 16] = 1.0 / 16.0
    indT = np.zeros((8, P), np.float32)
    for p in range(P):
        indT[p // 16, p] = 1.0
    return ind16, indT


def make_in_maps(inputs):
    x = np.asarray(inputs["x"], np.float32)
    ind16, indT = host_constants()
    wp = np.asarray(inputs["wp"], np.float32)
    bv = np.asarray(inputs["bv"], np.float32)
    # bv enters the output only as wp @ (bv * sum(attn)) = wp @ bv per query
    # (attention rows sum to 1), so fold it into the projection bias.
    bp_prime = np.asarray(inputs["bp"], np.float32) + wp @ bv
    shared = {
        "wqT": np.ascontiguousarray(np.asarray(inputs["wq"], np.float32).T),
        "wkT": np.ascontiguousarray(np.asarray(inputs["wk"], np.float32).T),
        "wvT": np.ascontiguousarray(np.asarray(inputs["wv"], np.float32).T),
        "wpT": np.ascontiguousarray(wp.T),
        "bq": np.asarray(inputs["bq"], np.float32),
        "bk": np.asarray(inputs["bk"], np.float32),
        "bp": bp_prime,
        "nscale": np.asarray(inputs["norm_scale"], np.float32),
        "nbias": np.asarray(inputs["norm_bias"], np.float32),
        "ind16": ind16, "indT": indT,
    }
    return [dict(shared, x=np.ascontiguousarray(x[i].reshape(C, HW)))
            for i in range(B)]


def kernel(**inputs):
    from concourse.bass_utils import run_bass_kernel_spmd

    if "nc" not in _CACHE:
        _CACHE["nc"] = build_nc()
    nc = _CACHE["nc"]
    in_maps = make_in_maps(inputs)
    res = run_bass_kernel_spmd(nc, in_maps, list(range(B)))
    out = np.stack([res.results[i]["out"] for i in range(B)])
    return out.reshape(B, C, 64, 64)


# revision 28
# speedup vs baseline: 1.0860x; 1.0154x over previous
"""AttnBlock2D Trainium2 kernel (8-core data-parallel over batch).

Per core: one batch element. x:[512, 4096] (c, h*w).
  h = GroupNorm32(x) * scale + bias
  q = wq@h, k = wk@h, v = wv@h          (1x1 conv == matmul over channels)
  attn = softmax(q^T k / sqrt(512));  out = v @ attn^T
  y = x + wp@out + bp'                  (bp' = bp + wp@bv, folded on host)

On-chip layout: S^T orientation ([keys on partitions, queries on free]) so the
softmax denominator is a partition-dim reduction done with a ones-matmul; the
PV contraction then needs no transposes at all (vT is produced directly by
swapping matmul operands). Softmax skips max-subtraction: scores are ~N(0,1)
by construction; exp() gets a -2 bias so fp8e4m3 outputs can't overflow (the
factor cancels between numerator and denominator).

All big matmuls run fp8e4m3 with perf_mode=DoubleRow: operands are stored as
[128, 2, n] (contraction = 128 partitions x 2 interleaved k-tiles), which
halves instruction count and PE streaming time vs bf16. PSUM stays fp32.
Attention-path quantization error is diluted ~40x in the output norm by the
residual, so rel-err stays ~5e-3, under the 2e-2 gate.

Softmax normalization is applied LATE (after the output projection):
y = x + (wp@U)*r + bp' with U the unnormalized PV output and r = 1/denominator
per query. This lets U leave PSUM as plain copies right after the last PV
(freeing the PV accumulator banks for the next query block immediately), and
the projection matmuls + normalize are emitted as pending stages interleaved
into the next query block's S-loop so the PE never waits on the recip chain.
"""
import os
import numpy as np

P = 128
C = 512
NCH = C // P              # 4 channel chunks
NC2 = NCH // 2            # 2 channel pair-chunks (DoubleRow)
HW = 4096                 # 64*64 pixels
QB = 512                  # query block (PSUM-bank limited)
NQB = HW // QB            # 8
NE = HW // P              # 32 key chunks
NE2 = NE // 2             # 16 key pair-chunks
NB = HW // 512            # 8 pixel blocks for projections
EPS = 1e-5
SCALE = 1.0 / np.sqrt(C)
# exp bias: scores are ~N(0,1), so exp(s - 3.39) keeps the fp8 probabilities
# and (more importantly) the unnormalized PV output U = sum(pt*v) safely
# below e4m3's 240 max while staying above the subnormal floor for all but
# a ~4% tail. The constant cancels between softmax numerator & denominator.
EXPB = -2.0 - float(np.log(4.0))
B = 8                     # batch / cores

_CACHE = {}


def _emit(nc, tc, ctx):
    import concourse.bass as bass
    from concourse import mybir
    from contextlib import ExitStack

    f32 = mybir.dt.float32
    bf16 = mybir.dt.bfloat16
    f8 = mybir.dt.float8e4
    AF = mybir.ActivationFunctionType
    OP = mybir.AluOpType
    DR = mybir.MatmulPerfMode.DoubleRow

    # ---------------- I/O ----------------
    x_d = nc.declare_dram_parameter("x", [C, HW], f32, isOutput=False).ap()
    wqT_d = nc.declare_dram_parameter("wqT", [C, C], f32, isOutput=False).ap()
    wkT_d = nc.declare_dram_parameter("wkT", [C, C], f32, isOutput=False).ap()
    wvT_d = nc.declare_dram_parameter("wvT", [C, C], f32, isOutput=False).ap()
    wpT_d = nc.declare_dram_parameter("wpT", [C, C], f32, isOutput=False).ap()
    bq_d = nc.declare_dram_parameter("bq", [C], f32, isOutput=False).ap()
    bk_d = nc.declare_dram_parameter("bk", [C], f32, isOutput=False).ap()
    bp_d = nc.declare_dram_parameter("bp", [C], f32, isOutput=False).ap()
    ns_d = nc.declare_dram_parameter("nscale", [C], f32, isOutput=False).ap()
    nb_d = nc.declare_dram_parameter("nbias", [C], f32, isOutput=False).ap()
    i16_d = nc.declare_dram_parameter("ind16", [P, 8], f32, isOutput=False).ap()
    iT_d = nc.declare_dram_parameter("indT", [8, P], f32, isOutput=False).ap()
    out_d = nc.declare_dram_parameter("out", [C, HW], f32, isOutput=True).ap()

    def col_ap(src, ci):
        # [128] slice of a [512] DRAM vector viewed as [128, 1]
        return bass.AP(tensor=src.tensor, offset=ci * P, ap=[[1, P], [0, 1]])

    # ---------------- persistent pools (allocated before phase-1a scope) ----
    cst = ctx.enter_context(tc.tile_pool(name="cst", bufs=1))
    wpT_sb = [cst.tile([P, 2, C], f8, name=f"wpT{i}", tag=f"wpT{i}")
              for i in range(NC2)]
    ind16_sb = cst.tile([P, 8], f32, name="ind16", tag="ind16")
    indT_sb = cst.tile([8, P], f32, name="indT", tag="indT")
    onec_sb = cst.tile([P, 1], bf16, name="onec", tag="onec")
    oner_sb = cst.tile([1, P], bf16, name="oner", tag="oner")
    bp_sb = [cst.tile([P, 1], f32, name=f"bp{i}", tag=f"bp{i}") for i in range(NCH)]
    expb_sb = cst.tile([P, 1], f32, name="expb", tag="expb")

    w_pool = tc.alloc_tile_pool(name="wqkv", bufs=1, side="right")
    wq2 = [w_pool.tile([P, 2, C], f8, name=f"wq2{i}", tag=f"wq2{i}") for i in range(NC2)]
    wk2 = [w_pool.tile([P, 2, C], f8, name=f"wk2{i}", tag=f"wk2{i}") for i in range(NC2)]
    wv2 = [w_pool.tile([P, 2, C], f8, name=f"wv2{i}", tag=f"wv2{i}") for i in range(NC2)]
    bq_sb = [w_pool.tile([P, 1], f32, name=f"bq{i}", tag=f"bq{i}") for i in range(NCH)]
    bk_sb = [w_pool.tile([P, 1], f32, name=f"bk{i}", tag=f"bk{i}") for i in range(NCH)]

    h_pool = tc.alloc_tile_pool(name="hres", bufs=1, side="right")
    h2 = [h_pool.tile([P, 2, HW], f8, name=f"h{i}", tag=f"h{i}") for i in range(NC2)]

    # ================ phase 1a: groupnorm (x DMA emitted first) ============
    with ExitStack() as s1:
        xp = s1.enter_context(tc.tile_pool(name="xp", bufs=1))
        wraw = s1.enter_context(tc.tile_pool(name="wraw", bufs=4))
        gn = s1.enter_context(tc.tile_pool(name="gn", bufs=2))
        gnp = s1.enter_context(tc.tile_pool(name="gnp", bufs=2, space="PSUM"))

        # x first: it gates the whole head (stats -> h -> qkv)
        x_sb = []
        for ci in range(NCH):
            t = xp.tile([P, HW], f32, name=f"x{ci}", tag=f"x{ci}")
            nc.sync.dma_start(out=t, in_=x_d[ci * P:(ci + 1) * P, :])
            x_sb.append(t)
        # small constants
        nc.sync.dma_start(out=ind16_sb, in_=i16_d)
        nc.sync.dma_start(out=indT_sb, in_=iT_d)
        nc.vector.memset(onec_sb, 1.0)
        nc.vector.memset(oner_sb, 1.0)
        nc.vector.memset(expb_sb, EXPB)
        nsc_sb, nbs_sb = [], []
        for ci in range(NCH):
            t = gn.tile([P, 1], f32, name=f"nsc{ci}", tag=f"nsc{ci}", bufs=1)
            nc.sync.dma_start(out=t, in_=col_ap(ns_d, ci))
            nsc_sb.append(t)
            t = gn.tile([P, 1], f32, name=f"nbs{ci}", tag=f"nbs{ci}", bufs=1)
            nc.sync.dma_start(out=t, in_=col_ap(nb_d, ci))
            nbs_sb.append(t)

        # per-chunk stats as x chunks land
        m2_all = gn.tile([P, 2 * NCH], f32, name="m2_all", tag="m2", bufs=1)
        for ci in range(NCH):
            stats = gn.tile([P, 8, 6], f32, name=f"st{ci}", tag="st")
            for s in range(8):
                nc.vector.bn_stats(out=stats[:, s, :],
                                   in_=x_sb[ci][:, s * 512:(s + 1) * 512])
            mv = gn.tile([P, 2], f32, name=f"mv{ci}", tag="mv")
            nc.vector.bn_aggr(out=mv, in_=stats)
            nc.vector.tensor_copy(out=m2_all[:, 2 * ci:2 * ci + 1], in_=mv[:, 0:1])
            msq = gn.tile([P, 1], f32, name=f"msq{ci}", tag="msq")
            nc.vector.tensor_mul(out=msq, in0=mv[:, 0:1], in1=mv[:, 0:1])
            nc.vector.tensor_add(out=m2_all[:, 2 * ci + 1:2 * ci + 2],
                                 in0=mv[:, 1:2], in1=msq)

        # k weights now (needed first in phase 1b); q/v/p stream later.
        # Only wk converts on ScalarE — everything else goes to VectorE so
        # the phase-1b k-activations aren't stuck behind a copy backlog.
        def load_w(srcd, dst2, tagp, eng="ve"):
            for ci in range(NCH):
                traw = wraw.tile([P, C], f32, name=f"{tagp}r{ci}", tag="wraw")
                nc.sync.dma_start(out=traw, in_=srcd[ci * P:(ci + 1) * P, :])
                if eng == "sc":
                    nc.scalar.copy(out=dst2[ci // 2][:, ci % 2, :], in_=traw)
                else:
                    nc.vector.tensor_copy(out=dst2[ci // 2][:, ci % 2, :],
                                          in_=traw)
        load_w(wkT_d, wk2, "wk", "sc")
        for i in range(NCH):
            nc.sync.dma_start(out=bk_sb[i], in_=col_ap(bk_d, i))
            nc.sync.dma_start(out=bq_sb[i], in_=col_ap(bq_d, i))
        nc.sync.dma_start(out=bp_sb[0], in_=col_ap(bp_d, 0))

        # group aggregation: 32 groups of 16 channels
        g_ps = gnp.tile([8, 2 * NCH], f32, name="g_ps", tag="gps")
        nc.tensor.matmul(g_ps, lhsT=ind16_sb, rhs=m2_all, start=True, stop=True)
        g_sb = gn.tile([8, 2 * NCH], f32, name="g_sb", tag="gsb", bufs=1)
        nc.vector.tensor_copy(out=g_sb, in_=g_ps)
        gv = g_sb.rearrange("p (c two) -> p c two", two=2)
        msq2 = gn.tile([8, NCH], f32, name="msq2", tag="msq2", bufs=1)
        nc.vector.tensor_mul(out=msq2, in0=gv[:, :, 0], in1=gv[:, :, 0])
        var_g = gn.tile([8, NCH], f32, name="var_g", tag="varg", bufs=1)
        nc.vector.tensor_sub(out=var_g, in0=gv[:, :, 1], in1=msq2)
        eps_t = gn.tile([8, 1], f32, name="eps_t", tag="eps", bufs=1)
        nc.vector.memset(eps_t, EPS)
        std_g = gn.tile([8, NCH], f32, name="std_g", tag="stdg", bufs=1)
        nc.scalar.activation(out=std_g, in_=var_g, func=AF.Sqrt,
                             bias=eps_t, scale=1.0)
        rstd_g = gn.tile([8, NCH], f32, name="rstd_g", tag="rstdg", bufs=1)
        nc.vector.reciprocal(out=rstd_g, in_=std_g)
        mr_g = gn.tile([8, NCH], f32, name="mr_g", tag="mrg", bufs=1)
        nc.vector.tensor_mul(out=mr_g, in0=gv[:, :, 0], in1=rstd_g)
        rb2 = gn.tile([8, 2 * NCH], f32, name="rb2", tag="rb2", bufs=1)
        rv = rb2.rearrange("p (c two) -> p c two", two=2)
        nc.vector.tensor_copy(out=rv[:, :, 0], in_=rstd_g)
        nc.vector.tensor_copy(out=rv[:, :, 1], in_=mr_g)

        for ci in range(NCH):
            ab_ps = gnp.tile([P, 2], f32, name=f"ab{ci}", tag="gps")
            nc.tensor.matmul(ab_ps, lhsT=indT_sb, rhs=rb2[:, 2 * ci:2 * ci + 2],
                             start=True, stop=True)
            A_t = gn.tile([P, 1], f32, name=f"A{ci}", tag="A")
            nc.vector.tensor_mul(out=A_t, in0=ab_ps[:, 0:1], in1=nsc_sb[ci])
            t0 = gn.tile([P, 1], f32, name=f"t0{ci}", tag="t0")
            nc.vector.tensor_mul(out=t0, in0=ab_ps[:, 1:2], in1=nsc_sb[ci])
            B_t = gn.tile([P, 1], f32, name=f"B{ci}", tag="Bt")
            nc.vector.tensor_sub(out=B_t, in0=nbs_sb[ci], in1=t0)
            # h = x*A + B, split across DVE and ScalarE to halve head latency
            if ci < 2:
                nc.vector.tensor_scalar(out=h2[ci // 2][:, ci % 2, :],
                                        in0=x_sb[ci],
                                        scalar1=A_t, scalar2=B_t,
                                        op0=OP.mult, op1=OP.add)
            else:
                nc.scalar.activation(out=h2[ci // 2][:, ci % 2, :],
                                     in_=x_sb[ci], func=AF.Identity,
                                     bias=B_t, scale=A_t)

        # remaining weights: emitted after h, converted on VectorE (execute
        # during phase 1b while DVE is otherwise idle)
        load_w(wqT_d, wq2, "wq")
        load_w(wvT_d, wv2, "wv")
        load_w(wpT_d, wpT_sb, "wp")
        for i in range(1, NCH):
            nc.sync.dma_start(out=bp_sb[i], in_=col_ap(bp_d, i))

    # q/k/vT fully SBUF-resident in fp8; allocated only now so phase 1a
    # could use this space for x
    vt_pool = ctx.enter_context(tc.tile_pool(name="vtres", bufs=1))
    vt2 = [vt_pool.tile([P, 2, C], f8, name=f"vt{e}", tag=f"vt{e}")
           for e in range(NE2)]
    k_pool = ctx.enter_context(tc.tile_pool(name="kres", bufs=1))
    k2 = [k_pool.tile([P, 2, HW], f8, name=f"k{i}", tag=f"k{i}")
          for i in range(NC2)]
    q_pool = ctx.enter_context(tc.tile_pool(name="qres", bufs=1))
    q2 = [q_pool.tile([P, 2, HW], f8, name=f"q{i}", tag=f"q{i}")
          for i in range(NC2)]

    # ================ phase 1b: k, q, vT -> resident sbuf (fp8 DR) ========
    # Paired pixel blocks ([P,2,512] PSUM tiles) halve the PSUM->SBUF
    # consumer instruction count; consumers are split across ScalarE (k)
    # and VectorE (q) with vT copies alternating between the two.
    with ExitStack() as s2:
        pp1 = s2.enter_context(tc.tile_pool(name="pp1", bufs=4, space="PSUM"))
        for nbp in range(NB // 2):
            for m in range(NCH):
                t = pp1.tile([P, 2, 512], f32, name="kps", tag="mm")
                for i in range(2):
                    nb = 2 * nbp + i
                    for c2 in range(NC2):
                        nc.tensor.matmul(t[:, i, :],
                                         lhsT=wk2[c2][:, :, m * P:(m + 1) * P],
                                         rhs=h2[c2][:, :, nb * 512:(nb + 1) * 512],
                                         start=(c2 == 0), stop=(c2 == NC2 - 1),
                                         perf_mode=DR)
                nc.scalar.activation(
                    out=k2[m // 2][:, m % 2, nbp * 1024:(nbp + 1) * 1024],
                    in_=t, func=AF.Identity, bias=bk_sb[m], scale=1.0)
        for nbp in range(NB // 2):
            for m in range(NCH):
                t = pp1.tile([P, 2, 512], f32, name="qps", tag="mm")
                for i in range(2):
                    nb = 2 * nbp + i
                    for c2 in range(NC2):
                        nc.tensor.matmul(t[:, i, :],
                                         lhsT=wq2[c2][:, :, m * P:(m + 1) * P],
                                         rhs=h2[c2][:, :, nb * 512:(nb + 1) * 512],
                                         start=(c2 == 0), stop=(c2 == NC2 - 1),
                                         perf_mode=DR)
                nc.vector.tensor_scalar_add(
                    out=q2[m // 2][:, m % 2, nbp * 1024:(nbp + 1) * 1024],
                    in0=t, scalar1=bq_sb[m])
        for e2 in range(NE2):
            t = pp1.tile([P, 2, 512], f32, name="vps", tag="mm")
            for j in range(2):
                e = 2 * e2 + j
                for c2 in range(NC2):
                    nc.tensor.matmul(t[:, j, :],
                                     lhsT=h2[c2][:, :, e * P:(e + 1) * P],
                                     rhs=wv2[c2],
                                     start=(c2 == 0), stop=(c2 == NC2 - 1),
                                     perf_mode=DR)
            # bv is folded into bp' on host, so vT is a plain copy;
            # alternate engines to balance load
            if e2 % 2 == 0:
                nc.scalar.copy(out=vt2[e2], in_=t)
            else:
                nc.vector.tensor_copy(out=vt2[e2], in_=t)

    h_pool.release()
    w_pool.release()

    # ================ phase 2: attention + proj ================
    with ExitStack() as s3:
        qxp = s3.enter_context(tc.tile_pool(name="qxp", bufs=8))
        ptp = s3.enter_context(tc.tile_pool(name="ptp", bufs=3))
        smp = s3.enter_context(tc.tile_pool(name="smp", bufs=2))
        osp = s3.enter_context(tc.tile_pool(name="osp", bufs=2))
        fnp = s3.enter_context(tc.tile_pool(name="fnp", bufs=2))
        pvp = s3.enter_context(tc.tile_pool(name="pvp", bufs=4, space="PSUM"))
        ssp = s3.enter_context(tc.tile_pool(name="ssp", bufs=3, space="PSUM"))
        smps = s3.enter_context(tc.tile_pool(name="smps", bufs=1, space="PSUM"))

        def make_stages(qb, osb2, dn_sb, xcur, last=False):
            hold = {}

            def st_rb():
                rb_ps = smps.tile([P, QB], f32, name="rb_ps", tag="sm")
                nc.tensor.matmul(rb_ps, lhsT=oner_sb, rhs=dn_sb,
                                 start=True, stop=True)
                rb_sb = smp.tile([P, QB], f32, name="rb_sb", tag="rbsb", bufs=2)
                nc.vector.reciprocal_approx_fast(out=rb_sb, in_=rb_ps)
                hold["rb"] = rb_sb

            def mk_pj(oc):
                def st_pj():
                    # the drain after the final query block has no S-loop to
                    # interleave with; use the freed PV banks so the four
                    # projections pipeline instead of serializing on one bank
                    pool, tag = (pvp, "pv") if last else (smps, "sm")
                    pj_ps = pool.tile([P, QB], f32, name="pj_ps", tag=tag)
                    for c2 in range(NC2):
                        nc.tensor.matmul(pj_ps,
                                         lhsT=wpT_sb[c2][:, :, oc * P:(oc + 1) * P],
                                         rhs=osb2[c2],
                                         start=(c2 == 0), stop=(c2 == NC2 - 1),
                                         perf_mode=DR)
                    t1 = fnp.tile([P, QB], f32, name="t1", tag="t1", bufs=2)
                    nc.vector.tensor_mul(out=t1, in0=pj_ps, in1=hold["rb"])
                    fin = fnp.tile([P, QB], f32, name="fin", tag="fin", bufs=2)
                    nc.vector.scalar_tensor_tensor(out=fin, in0=t1,
                                                   scalar=bp_sb[oc],
                                                   in1=xcur[oc],
                                                   op0=OP.add, op1=OP.add)
                    nc.sync.dma_start(out=out_d[oc * P:(oc + 1) * P,
                                                qb * QB:(qb + 1) * QB], in_=fin)
                return st_pj

            return [st_rb, mk_pj(0), mk_pj(1), mk_pj(2), mk_pj(3)]

        stages = []
        for qb in range(NQB):
            xcur = []
            for ci in range(NCH):
                t = qxp.tile([P, QB], f32, name=f"xb{ci}", tag="xblk")
                nc.sync.dma_start(out=t, in_=x_d[ci * P:(ci + 1) * P,
                                                 qb * QB:(qb + 1) * QB])
                xcur.append(t)
            acc2 = smp.tile([P, 2, QB], bf16, name="acc2", tag="acc")
            pvs = None
            for e2 in range(NE2):
                pt2 = ptp.tile([P, 2, QB], f8, name="pt", tag="pt")
                for j in range(2):
                    e = 2 * e2 + j
                    s_ps = ssp.tile([P, QB], f32, name="s_ps", tag="s")
                    for c2 in range(NC2):
                        nc.tensor.matmul(s_ps,
                                         lhsT=k2[c2][:, :, e * P:(e + 1) * P],
                                         rhs=q2[c2][:, :, qb * QB:(qb + 1) * QB],
                                         start=(c2 == 0), stop=(c2 == NC2 - 1),
                                         perf_mode=DR)
                    nc.scalar.activation(out=pt2[:, j, :], in_=s_ps, func=AF.Exp,
                                         bias=expb_sb, scale=float(SCALE))
                # paired accumulate (both j lanes in one DVE op)
                if e2 == 0:
                    nc.vector.tensor_copy(out=acc2, in_=pt2)
                else:
                    nc.vector.tensor_add(out=acc2, in0=acc2, in1=pt2)
                # previous qb's projection work, interleaved so the PE
                # never waits on the recip chain
                if stages:
                    stages.pop(0)()
                if e2 == 0:
                    pvs = [pvp.tile([P, QB], f32, name=f"pv{co}", tag="pv")
                           for co in range(NCH)]
                for co in range(NCH):
                    nc.tensor.matmul(pvs[co],
                                     lhsT=vt2[e2][:, :, co * P:(co + 1) * P],
                                     rhs=pt2,
                                     start=(e2 == 0), stop=(e2 == NE2 - 1),
                                     perf_mode=DR)
            # fold the two acc lanes, then denominator matmul
            accf = smp.tile([P, QB], bf16, name="accf", tag="accf", bufs=2)
            nc.vector.tensor_add(out=accf, in0=acc2[:, 0, :], in1=acc2[:, 1, :])
            dnrb = smps.tile([P, QB], f32, name="dnrb", tag="sm")
            nc.tensor.matmul(dnrb[0:1, :], lhsT=onec_sb, rhs=accf,
                             start=True, stop=True)
            dn_sb = smp.tile([1, QB], bf16, name="dn_sb", tag="dnsb", bufs=2)
            nc.scalar.copy(out=dn_sb, in_=dnrb[0:1, :])
            osb2 = [osp.tile([P, 2, QB], f8, name=f"osb{c2}", tag=f"osb{c2}",
                             bufs=2) for c2 in range(NC2)]
            for co in range(NCH):
                if co < 2:
                    nc.scalar.copy(out=osb2[co // 2][:, co % 2, :], in_=pvs[co])
                else:
                    nc.vector.tensor_copy(out=osb2[co // 2][:, co % 2, :],
                                          in_=pvs[co])
            stages = make_stages(qb, osb2, dn_sb, xcur, last=(qb == NQB - 1))
        while stages:
            stages.pop(0)()


def build_nc():
    import concourse.bacc as bacc
    import concourse.tile as tile
    from contextlib import ExitStack

    nc = bacc.Bacc("TRN2", target_bir_lowering=False, debug=False)
    with tile.TileContext(nc) as tc:
        with ExitStack() as ctx:
            _emit(nc, tc, ctx)
    nc.finalize()
    return nc


def host_constants():
    ind16 = np.zeros((P, 8), np.float32)
    for p in range(P):
        ind16[p, p // 16] = 1.0 / 16.0
    indT = np.zeros((8, P), np.float32)
    for p in range(P):
        indT[p // 16, p] = 1.0
    return ind16, indT


def make_in_maps(inputs):
    x = np.asarray(inputs["x"], np.float32)
    ind16, indT = host_constants()
    wp = np.asarray(inputs["wp"], np.float32)
    bv = np.asarray(inputs["bv"], np.float32)
    # bv enters the output only as wp @ (bv * sum(attn)) = wp @ bv per query
    # (attention rows sum to 1), so fold it into the projection bias.
    bp_prime = np.asarray(inputs["bp"], np.float32) + wp @ bv
    shared = {
        "wqT": np.ascontiguousarray(np.asarray(inputs["wq"], np.float32).T),
        "wkT": np.ascontiguousarray(np.asarray(inputs["wk"], np.float32).T),
        "wvT": np.ascontiguousarray(np.asarray(inputs["wv"], np.float32).T),
        "wpT": np.ascontiguousarray(wp.T),
        "bq": np.asarray(inputs["bq"], np.float32),
        "bk": np.asarray(inputs["bk"], np.float32),
        "bp": bp_prime,
        "nscale": np.asarray(inputs["norm_scale"], np.float32),
        "nbias": np.asarray(inputs["norm_bias"], np.float32),
        "ind16": ind16, "indT": indT,
    }
    return [dict(shared, x=np.ascontiguousarray(x[i].reshape(C, HW)))
            for i in range(B)]


def kernel(**inputs):
    from concourse.bass_utils import run_bass_kernel_spmd

    if "nc" not in _CACHE:
        _CACHE["nc"] = build_nc()
    nc = _CACHE["nc"]
    in_maps = make_in_maps(inputs)
    res = run_bass_kernel_spmd(nc, in_maps, list(range(B)))
    out = np.stack([res.results[i]["out"] for i in range(B)])
    return out.reshape(B, C, 64, 64)
